# revision 2
# baseline (speedup 1.0000x reference)
"""Trainium2 Bass kernel for causal self-attention (nn_CausalSelfAttention).

Sharding: tensor-parallel on heads + data-parallel on batch.
8 cores = 2 batches x 4 head-groups (4 heads of 64 dims each per core).

v2 (merged-phase) design, all matmuls bf16 (full PE rate, halves DMA):
  - Single phase: projections STREAM into the attention pipeline. Up
    front: Q dt0 + K dt0 (heads 0/1) + V t-blocks 0-3; the remaining
    projections (V 4-15, Q dt1, K dt1) are paced one unit per score
    slot across heads 0-2 so the PE never drains and exp starts ~22us
    earlier than a two-phase structure allows.
  - Scores s^T[key, query] per 128-key strip into 2-bank PSUM tiles,
    exp on ScalarE (unstabilized: |scores| <= ~8), strips packed into
    a per-head [128, 17408] SBUF buffer (2 buffers, alternating heads).
  - Causal mask multiply of the diagonal 128-block on GpSimd (DVE was
    a co-bottleneck at 103us busy in v1).
  - attV as ONE-SHOT bursts per (head, 512-query block): accumulate
    over all key strips in a single [65,512] PSUM tile (V carries a
    ones column so the softmax denominator rides along). Only 2 attV
    PSUM tiles live at any time -> total PSUM = 2x[128,512] proj +
    2x[128,1024] scores + 2x[65,512] attV = exactly 8 banks.
  - Normalization per block right after its burst (denom row copy ->
    GpSimd partition broadcast -> fast reciprocal -> scale), so y
    blocks finalize progressively.
  - Output projection per y-block as soon as the 4th head closes it
    (spread through head 3's score slots); PSUM->SBUF casts to bf16
    split 3:1 DVE:ACT; bf16 output DMA. Host sums the 4 TP partials
    per batch and adds the bias.
"""
import sys

if "/opt/trn_rl_repo" not in sys.path:
    sys.path.insert(0, "/opt/trn_rl_repo")

import ml_dtypes
import numpy as np

import concourse.bacc as bacc
import concourse.mybir as mybir
from concourse.bass import _add_dep_helper
import concourse.tile as tile
from concourse.bass_utils import run_bass_kernel_spmd

B, T, C, H, D = 2, 2048, 1024, 16, 64
NCORES = 8
HPC = H // (NCORES // B)  # 4 heads per core
CS = HPC * D              # 256 channel-shard
P = 128
CT = C // P               # 8 contraction tiles
DT = CS // P              # 2 d-tiles for q
NTB = T // P              # 16 key strips of 128
F32 = mybir.dt.float32
BF16 = mybir.dt.bfloat16
EXP = mybir.ActivationFunctionType.Exp

# packed strip buffer layout: strip jb is [128, W_JB[jb]] at col OFF_JB[jb]
W_JB = [T - P * jb for jb in range(NTB)]
OFF_JB = [0] * NTB
for _jb in range(1, NTB):
    OFF_JB[_jb] = OFF_JB[_jb - 1] + W_JB[_jb - 1]
ATTW = OFF_JB[-1] + W_JB[-1]  # 17408

LAST_RESULTS = None  # BassKernelResults of the most recent kernel() call


def _exp_tiles(W):
    """Split [0, W) into PSUM-tile pieces for the scores matmuls + exp.
    Each piece is a list of matmul chunks (off, w<=512) that land in one
    2-bank PSUM tile; chunk k sits at bank offset 512*k so only the last
    chunk may be partial (keeps the exp read contiguous)."""
    pieces = []
    off = 0
    while off < W:
        rem = W - off
        if rem > 512:
            w2 = min(512, rem - 512)
            pieces.append([(off, 512), (off + 512, w2)])
            off += 512 + w2
        else:
            pieces.append([(off, rem)])
            off += rem
    return pieces


def _emit(nc, tc):
    xT = nc.dram_tensor("xT", [C, T], BF16, kind="ExternalInput").ap()
    wqT = nc.dram_tensor("wqT", [C, CS], BF16, kind="ExternalInput").ap()
    wkT = nc.dram_tensor("wkT", [C, CS], BF16, kind="ExternalInput").ap()
    wvT = nc.dram_tensor("wvT", [C, CS], BF16, kind="ExternalInput").ap()
    wpT = nc.dram_tensor("wpT", [CS, C], BF16, kind="ExternalInput").ap()
    mask = nc.dram_tensor("mask", [P, P], BF16, kind="ExternalInput").ap()
    out = nc.dram_tensor("out", [T, C], BF16, kind="ExternalOutput").ap()

    with (
        tc.tile_pool(name="sb", bufs=1) as pp,
        tc.tile_pool(name="ps", bufs=1, space="PSUM") as pq,
    ):
        qT = pp.tile([P, DT, T], BF16, name="qT")
        # zero-padded per-head K^T: head h's 64 rows live at partition
        # offset 64*(h%2); the other 64 partitions are zero so the scores
        # matmul can contract over the full 128-partition interleaved qT.
        kz = [pp.tile([P, T], BF16, name=f"kz{h}") for h in range(HPC)]
        vp = pp.tile([P, NTB, HPC, D + 1], BF16, name="vp")
        yT = pp.tile([P, DT, T], BF16, name="yT")
        wp_sb = pp.tile([P, DT, C], BF16, name="wp_sb")
        mask_sb = pp.tile([P, P], BF16, name="mask_sb")
        att = [pp.tile([P, ATTW], BF16, name=f"att{i}") for i in range(2)]

        # ---------------- input DMAs ----------------
        w_sbs = {}
        for nm, dram in (("wq", wqT), ("wk", wkT), ("wv", wvT)):
            w_sb = pp.tile([P, CT, CS], BF16, name=f"{nm}_sb")
            nc.sync.dma_start(w_sb, dram.rearrange("(o p) c -> p o c", p=P))
            w_sbs[nm] = w_sb
        xTr = xT.rearrange("(co p) t -> p co t", p=P)
        xc = []
        xdma = []
        for i in range(4):
            xt = pp.tile([P, CT, 512], BF16, name=f"xc{i}")
            xdma.append(
                nc.gpsimd.dma_start(xt, xTr[:, :, i * 512:(i + 1) * 512])
            )
            xc.append(xt)
        nc.sync.dma_start(wp_sb, wpT.rearrange("(o p) c -> p o c", p=P))
        nc.sync.dma_start(mask_sb, mask)

        # zero the dead half of each kz tile; ones columns of V'
        for h in range(HPC):
            ro = D * (h % 2)
            dead = 0 if ro else D
            nc.vector.memset(kz[h][dead:dead + D, :], 0.0)
        nc.vector.memset(vp[:, :, :, D], 1.0)

        # warm the EXP activation table while the ACT engine is idle
        warm = pp.tile([1, 2], F32, name="warm")
        nc.vector.memset(warm, 0.0)
        nc.scalar.activation(warm, warm, EXP)

        # dummy broadcast: loads the GpSimd ISA library (~7us) during the
        # projection lead-in (the mask multiplies need it); held past the
        # x DMAs so the library-code DMA doesn't steal HBM bandwidth
        libw = pp.tile([2, 2], F32, name="libw")
        _lw = nc.gpsimd.partition_broadcast(libw, warm[0:1, :])
        _add_dep_helper(_lw.ins, xdma[3].ins, sync=True,
                        reason="delay gpsimd lib load past x DMAs")

        # ---------------- emission units ----------------
        def proj_kq(kind, dt_, tb):
            ts_ = slice(tb * 512, (tb + 1) * 512)
            ps = pq.tile([P, 512], F32, tag="pp", bufs=2, name="pps")
            for ct in range(CT):
                nc.tensor.matmul(
                    ps,
                    lhsT=w_sbs[kind][:, ct, dt_ * P:(dt_ + 1) * P],
                    rhs=xc[tb][:, ct, :],
                    start=(ct == 0),
                    stop=(ct == CT - 1),
                )
            if kind == "wq":
                nc.vector.tensor_copy(qT[:, dt_, ts_], ps)
            else:
                # K psum rows [0:64] belong to head 2*dt_, [64:128] to
                # head 2*dt_+1; scatter into the zero-padded kz tiles
                nc.vector.tensor_copy(kz[2 * dt_][0:D, ts_], ps[0:D, :])
                nc.vector.tensor_copy(kz[2 * dt_ + 1][D:P, ts_], ps[D:P, :])

        def proj_v(tb):
            ps = pq.tile([P, 512], F32, tag="pp", bufs=2, name="pps")
            for ct in range(CT):
                nc.tensor.matmul(
                    ps[:, 0:CS],
                    lhsT=xc[tb // 4][:, ct, (tb % 4) * P:(tb % 4 + 1) * P],
                    rhs=w_sbs["wv"][:, ct, :],
                    start=(ct == 0),
                    stop=(ct == CT - 1),
                )
            nc.vector.tensor_copy(
                vp[:, tb, :, 0:D],
                ps[:, 0:CS].rearrange("p (h d) -> p h d", h=HPC),
            )

        def run_unit(u):
            if u[0] == "kq":
                proj_kq(u[1], u[2], u[3])
            else:
                proj_v(u[1])

        def scores(h, jb):
            dt_ = h // 2
            j0 = jb * P
            Wj = W_JB[jb]
            strip = att[h % 2][:, OFF_JB[jb]:OFF_JB[jb] + Wj]
            for piece in _exp_tiles(Wj):
                pw = piece[-1][0] + piece[-1][1] - piece[0][0]
                ps = pq.tile([P, 1024], F32, tag="sps", bufs=2, name="sps")
                for k, (coff, cw) in enumerate(piece):
                    nc.tensor.matmul(
                        ps[:, k * 512:k * 512 + cw],
                        lhsT=kz[h][:, j0:j0 + P],
                        rhs=qT[:, dt_, j0 + coff:j0 + coff + cw],
                        start=True,
                        stop=True,
                    )
                p0 = piece[0][0]
                nc.scalar.activation(strip[:, p0:p0 + pw], ps[:, 0:pw], EXP)
            # causal mask on the diagonal 128-block (GpSimd, off DVE)
            nc.gpsimd.tensor_mul(
                out=strip[:, 0:P], in0=strip[:, 0:P], in1=mask_sb
            )

        def burst(h, ib):
            """One-shot attV for y-block ib of head h: accumulate over all
            key strips jb<=4*ib+3 in a single [65,512] PSUM tile, then
            normalize into yT."""
            dt_ = h // 2
            ro = D * (h % 2)
            py = pq.tile([D + 1, 512], F32, tag="yps", bufs=2, name="yps")
            last_jb = 4 * ib + 3
            for jb in range(last_jb + 1):
                lo_g = max(512 * ib, P * jb)
                hi_g = 512 * (ib + 1)
                nc.tensor.matmul(
                    py[:, lo_g - 512 * ib:hi_g - 512 * ib],
                    lhsT=vp[:, jb, h, :],
                    rhs=att[h % 2][:, OFF_JB[jb] + lo_g - P * jb:
                                   OFF_JB[jb] + hi_g - P * jb],
                    start=(jb == 0),
                    stop=(jb == last_jb),
                    skip_group_check=True,
                )
            # denominator row -> SBUF, broadcast across 64 partitions on
            # GpSimd, fast reciprocal (~18 bits), scale y^T out of PSUM
            srow = pp.tile([1, 512], F32, tag="srow", bufs=2, name="srow")
            nc.vector.tensor_copy(srow, py[D:D + 1, :])
            sbc = pp.tile([D, 512], F32, tag="sbc", bufs=2, name="sbc")
            nc.gpsimd.partition_broadcast(sbc, srow)
            rsb = pp.tile([D, 512], F32, tag="rsb", bufs=2, name="rsb")
            nc.vector.reciprocal_approx_fast(out=rsb, in_=sbc)
            nc.vector.tensor_mul(
                out=yT[ro:ro + D, dt_, 512 * ib:512 * (ib + 1)],
                in0=py[0:D, :],
                in1=rsb,
            )

        def outproj_chunk(ib, which):
            """Project+store 256 output rows of y-block ib (2 chunks/block)."""
            tbp = 2 * ib + which
            osb = pp.tile([P, 2, C], BF16, tag="osb", bufs=3, name="osb")
            for g in range(2):
                tb = 2 * tbp + g
                for ob in range(2):
                    ps = pq.tile([P, 512], F32, tag="pp", bufs=2, name="ops")
                    for ct2 in range(DT):
                        nc.tensor.matmul(
                            ps,
                            lhsT=yT[:, ct2, tb * P:(tb + 1) * P],
                            rhs=wp_sb[:, ct2, ob * 512:(ob + 1) * 512],
                            start=(ct2 == 0),
                            stop=(ct2 == DT - 1),
                        )
                    dst = osb[:, g, ob * 512:(ob + 1) * 512]
                    if g == 1 and ob == 1:
                        nc.scalar.copy(dst, ps)  # 1/4 of casts on ACT
                    else:
                        nc.vector.tensor_copy(dst, ps)
            nc.sync.dma_start(
                out[tbp * 256:(tbp + 1) * 256, :]
                .rearrange("(g p) c -> p g c", p=P),
                osb,
            )

        # ---------------- master schedule ----------------
        # lead-in: everything attention h0 needs (Q dt0 full, K dt0 full)
        # plus V t-blocks 0-3, consumption ordered to match x-chunk DMAs
        pre = [
            ("kq", "wq", 0, 0), ("kq", "wk", 0, 0),
            ("v", 0), ("v", 1), ("v", 2), ("v", 3),
            ("kq", "wq", 0, 1), ("kq", "wk", 0, 1),
            ("kq", "wq", 0, 2), ("kq", "wk", 0, 2),
            ("kq", "wq", 0, 3), ("kq", "wk", 0, 3),
        ]
        for u in pre:
            run_unit(u)

        # remaining projection units, paced into the score slots of each
        # head pass (keeps the PE fed while ACT works through the exps)
        inter = {
            0: [("v", tb) for tb in range(4, NTB)],
            1: [("kq", "wq", 1, tb) for tb in range(4)]
               + [("kq", "wk", 1, 0)],
            2: [("kq", "wk", 1, 1), ("kq", "wk", 1, 2), ("kq", "wk", 1, 3)],
            3: [],
        }
        for h in range(HPC):
            units = inter[h]
            stride = (NTB // len(units)) if units else 0
            ui = 0
            for jb in range(NTB):
                if units and ui < len(units) and stride and jb % stride == 0:
                    run_unit(units[ui])
                    ui += 1
                scores(h, jb)
                # bursts lag one strip so the exp of their last strip has
                # PE work (next scores) to hide behind
                if jb in (4, 8, 12):
                    ib = jb // 4 - 1
                    burst(h, ib)
                    if h == HPC - 1:
                        outproj_chunk(ib, 0)
                elif jb in (5, 9, 13) and h == HPC - 1:
                    outproj_chunk(jb // 4 - 1, 1)
            burst(h, 3)
            if h == HPC - 1:
                outproj_chunk(3, 0)
                outproj_chunk(3, 1)


def build_program(num_devices=NCORES):
    nc = bacc.Bacc(
        "TRN2",
        target_bir_lowering=False,
        debug=False,
        num_devices=num_devices,
    )
    with tile.TileContext(nc) as tc:
        _emit(nc, tc)
    nc.compile()
    return nc


_PROGRAM = None


def _get_program():
    global _PROGRAM
    if _PROGRAM is None:
        _PROGRAM = build_program()
    return _PROGRAM


def make_in_maps(x, Wk, Wq, Wv, Wp):
    bf = ml_dtypes.bfloat16
    mask = np.triu(np.ones((P, P), np.float32)).astype(bf)
    in_maps = []
    for core in range(NCORES):
        b, g = divmod(core, HPC)
        rows = slice(CS * g, CS * (g + 1))
        in_maps.append({
            "xT": np.ascontiguousarray(x[b].T).astype(bf),
            "wqT": (np.ascontiguousarray(Wq[rows].T)
                    * np.float32(0.125)).astype(bf),
            "wkT": np.ascontiguousarray(Wk[rows].T).astype(bf),
            "wvT": np.ascontiguousarray(Wv[rows].T).astype(bf),
            "wpT": np.ascontiguousarray(Wp[:, rows].T).astype(bf),
            "mask": mask,
        })
    return in_maps


def kernel(x, Wk, Wq, Wv, Wp, bp):
    global LAST_RESULTS
    x = np.asarray(x, dtype=np.float32)
    Wk = np.asarray(Wk, dtype=np.float32)
    Wq = np.asarray(Wq, dtype=np.float32)
    Wv = np.asarray(Wv, dtype=np.float32)
    Wp = np.asarray(Wp, dtype=np.float32)
    bp = np.asarray(bp, dtype=np.float32)

    nc = _get_program()
    res = run_bass_kernel_spmd(
        nc, make_in_maps(x, Wk, Wq, Wv, Wp), core_ids=list(range(NCORES))
    )
    LAST_RESULTS = res

    out = np.zeros((B, T, C), np.float64)
    for core in range(NCORES):
        out[core // HPC] += res.results[core]["out"].astype(np.float64)
    out += bp.astype(np.float64)[None, None, :]
    return out.astype(np.float32)


# revision 6
# speedup vs baseline: 2.2958x; 2.2958x over previous
"""Trainium2 Bass kernel for causal self-attention (nn_CausalSelfAttention).

Sharding: tensor-parallel on heads + data-parallel on batch.
8 cores = 2 batches x 4 head-groups (4 heads of 64 dims each per core).

v2 (merged-phase) design, all matmuls bf16 (full PE rate, halves DMA):
  - Single phase: projections STREAM into the attention pipeline. Up
    front: Q dt0 + K dt0 (heads 0/1) + V t-blocks 0-3; the remaining
    projections (V 4-15, Q dt1, K dt1) are paced one unit per score
    slot across heads 0-2 so the PE never drains and exp starts ~22us
    earlier than a two-phase structure allows.
  - Scores s^T[key, query] per 128-key strip into 2-bank PSUM tiles,
    exp on ScalarE (unstabilized: |scores| <= ~8), strips packed into
    a per-head [128, 17408] SBUF buffer (2 buffers, alternating heads).
  - Causal mask multiply of the diagonal 128-block on GpSimd (DVE was
    a co-bottleneck at 103us busy in v1).
  - attV as ONE-SHOT bursts per (head, 512-query block): accumulate
    over all key strips in a single [65,512] PSUM tile (V carries a
    ones column so the softmax denominator rides along). Only 2 attV
    PSUM tiles live at any time -> total PSUM = 2x[128,512] proj +
    2x[128,1024] scores + 2x[65,512] attV = exactly 8 banks.
  - Normalization per block right after its burst (denom row copy ->
    GpSimd partition broadcast -> fast reciprocal -> scale), so y
    blocks finalize progressively.
  - Output projection per y-block as soon as the 4th head closes it
    (spread through head 3's score slots); PSUM->SBUF casts to bf16
    split 3:1 DVE:ACT; bf16 output DMA. Host sums the 4 TP partials
    per batch and adds the bias.
"""
import sys

if "/opt/trn_rl_repo" not in sys.path:
    sys.path.insert(0, "/opt/trn_rl_repo")

import ml_dtypes
import numpy as np

import concourse.bacc as bacc
import concourse.mybir as mybir
from concourse.bass import _add_dep_helper
import concourse.tile as tile
from concourse.bass_utils import run_bass_kernel_spmd

B, T, C, H, D = 2, 2048, 1024, 16, 64
NCORES = 8
HPC = H // (NCORES // B)  # 4 heads per core
CS = HPC * D              # 256 channel-shard
P = 128
CT = C // P               # 8 contraction tiles
DT = CS // P              # 2 d-tiles for q
NTB = T // P              # 16 key strips of 128
F32 = mybir.dt.float32
BF16 = mybir.dt.bfloat16
EXP = mybir.ActivationFunctionType.Exp

# packed strip buffer layout: strip jb is [128, W_JB[jb]] at col OFF_JB[jb]
W_JB = [T - P * jb for jb in range(NTB)]
OFF_JB = [0] * NTB
for _jb in range(1, NTB):
    OFF_JB[_jb] = OFF_JB[_jb - 1] + W_JB[_jb - 1]
ATTW = OFF_JB[-1] + W_JB[-1]  # 17408

LAST_RESULTS = None  # BassKernelResults of the most recent kernel() call


def _exp_tiles(W):
    """Split [0, W) into PSUM-tile pieces for the scores matmuls + exp.
    Each piece is a list of matmul chunks (off, w<=512) that land in one
    2-bank PSUM tile; chunk k sits at bank offset 512*k so only the last
    chunk may be partial (keeps the exp read contiguous)."""
    pieces = []
    off = 0
    while off < W:
        rem = W - off
        if rem > 512:
            w2 = min(512, rem - 512)
            pieces.append([(off, 512), (off + 512, w2)])
            off += 512 + w2
        else:
            pieces.append([(off, rem)])
            off += rem
    return pieces


def _emit(nc, tc):
    xT = nc.dram_tensor("xT", [C, T], BF16, kind="ExternalInput").ap()
    wqT = nc.dram_tensor("wqT", [C, CS], BF16, kind="ExternalInput").ap()
    wkT = nc.dram_tensor("wkT", [C, CS], BF16, kind="ExternalInput").ap()
    wvT = nc.dram_tensor("wvT", [C, CS], BF16, kind="ExternalInput").ap()
    wpT = nc.dram_tensor("wpT", [CS, C], BF16, kind="ExternalInput").ap()
    mask = nc.dram_tensor("mask", [P, P], BF16, kind="ExternalInput").ap()
    out = nc.dram_tensor("out", [T, C], BF16, kind="ExternalOutput").ap()

    with (
        tc.tile_pool(name="sb", bufs=1) as pp,
        tc.tile_pool(name="ps", bufs=1, space="PSUM") as pq,
    ):
        qT = pp.tile([P, DT, T], BF16, name="qT")
        # zero-padded per-head K^T: head h's 64 rows live at partition
        # offset 64*(h%2); the other 64 partitions are zero so the scores
        # matmul can contract over the full 128-partition interleaved qT.
        kz = [pp.tile([P, T], BF16, name=f"kz{h}") for h in range(HPC)]
        vp = pp.tile([P, NTB, HPC, D + 1], BF16, name="vp")
        yT = pp.tile([P, DT, T], BF16, name="yT")
        wp_sb = pp.tile([P, DT, C], BF16, name="wp_sb")
        mask_sb = pp.tile([P, P], BF16, name="mask_sb")
        att = [pp.tile([P, ATTW], BF16, name=f"att{i}") for i in range(2)]

        # ---------------- input DMAs ----------------
        # DMAs spray across all 16 HW engines, so concurrent transfers
        # share the ~358 GB/s and ALL finish late. Chain each queue so
        # bytes arrive in consumption order: wq then wk/wv/wp on sync,
        # xc0..xc3 on gpsimd; the two chains run in parallel.
        w_sbs = {}
        wdma = []
        for nm, dram in (("wq", wqT), ("wk", wkT), ("wv", wvT)):
            w_sb = pp.tile([P, CT, CS], BF16, name=f"{nm}_sb")
            wdma.append(
                nc.sync.dma_start(w_sb, dram.rearrange("(o p) c -> p o c", p=P))
            )
            w_sbs[nm] = w_sb
        wdma.append(
            nc.sync.dma_start(wp_sb, wpT.rearrange("(o p) c -> p o c", p=P))
        )
        wdma.append(nc.sync.dma_start(mask_sb, mask))
        for i in range(1, len(wdma)):
            _add_dep_helper(wdma[i].ins, wdma[i - 1].ins, sync=True,
                            reason="serialize weight DMAs in consumption order")
        xTr = xT.rearrange("(co p) t -> p co t", p=P)
        xc = []
        xdma = []
        for i in range(4):
            xt = pp.tile([P, CT, 512], BF16, name=f"xc{i}")
            xdma.append(
                nc.gpsimd.dma_start(xt, xTr[:, :, i * 512:(i + 1) * 512])
            )
            xc.append(xt)
        for i in range(1, 4):
            _add_dep_helper(xdma[i].ins, xdma[i - 1].ins, sync=True,
                            reason="serialize x DMAs in consumption order")

        # warm the EXP activation table while the ACT engine is idle
        warm = pp.tile([1, 2], F32, name="warm")
        nc.vector.memset(warm, 0.0)
        nc.scalar.activation(warm, warm, EXP)

        # dummy broadcast: loads the GpSimd ISA library (~7us) during the
        # projection lead-in (the mask multiplies need it); held past the
        # x DMAs so the library-code DMA doesn't steal HBM bandwidth
        libw = pp.tile([2, 2], F32, name="libw")
        _lw = nc.gpsimd.partition_broadcast(libw, warm[0:1, :])
        _add_dep_helper(_lw.ins, xdma[3].ins, sync=True,
                        reason="delay gpsimd lib load past x DMAs")

        # ---------------- emission units ----------------
        def proj_kq(kind, dt_, tb):
            ts_ = slice(tb * 512, (tb + 1) * 512)
            ps = pq.tile([P, 512], F32, tag="pp", bufs=2, name="pps")
            for ct in range(CT):
                nc.tensor.matmul(
                    ps,
                    lhsT=w_sbs[kind][:, ct, dt_ * P:(dt_ + 1) * P],
                    rhs=xc[tb][:, ct, :],
                    start=(ct == 0),
                    stop=(ct == CT - 1),
                )
            if kind == "wq":
                nc.vector.tensor_copy(qT[:, dt_, ts_], ps)
            else:
                # K psum rows [0:64] belong to head 2*dt_, [64:128] to
                # head 2*dt_+1; scatter into the zero-padded kz tiles
                nc.vector.tensor_copy(kz[2 * dt_][0:D, ts_], ps[0:D, :])
                nc.vector.tensor_copy(kz[2 * dt_ + 1][D:P, ts_], ps[D:P, :])

        def proj_v(tb):
            ps = pq.tile([P, 512], F32, tag="pp", bufs=2, name="pps")
            for ct in range(CT):
                nc.tensor.matmul(
                    ps[:, 0:CS],
                    lhsT=xc[tb // 4][:, ct, (tb % 4) * P:(tb % 4 + 1) * P],
                    rhs=w_sbs["wv"][:, ct, :],
                    start=(ct == 0),
                    stop=(ct == CT - 1),
                )
            nc.vector.tensor_copy(
                vp[:, tb, :, 0:D],
                ps[:, 0:CS].rearrange("p (h d) -> p h d", h=HPC),
            )

        def run_unit(u):
            if u[0] == "kq":
                proj_kq(u[1], u[2], u[3])
            else:
                proj_v(u[1])

        def scores(h, jb):
            dt_ = h // 2
            j0 = jb * P
            Wj = W_JB[jb]
            strip = att[h % 2][:, OFF_JB[jb]:OFF_JB[jb] + Wj]
            for piece in _exp_tiles(Wj):
                pw = piece[-1][0] + piece[-1][1] - piece[0][0]
                ps = pq.tile([P, 1024], F32, tag="sps", bufs=2, name="sps")
                for k, (coff, cw) in enumerate(piece):
                    nc.tensor.matmul(
                        ps[:, k * 512:k * 512 + cw],
                        lhsT=kz[h][:, j0:j0 + P],
                        rhs=qT[:, dt_, j0 + coff:j0 + coff + cw],
                        start=True,
                        stop=True,
                    )
                p0 = piece[0][0]
                nc.scalar.activation(strip[:, p0:p0 + pw], ps[:, 0:pw], EXP)
            # causal mask on the diagonal 128-block. Must stay on DVE:
            # GpSimd tensor ops live in a different ISA library than
            # partition_broadcast and each switch costs ~6.5us UNLOAD_LIB
            nc.vector.tensor_mul(
                out=strip[:, 0:P], in0=strip[:, 0:P], in1=mask_sb
            )

        def burst(h, ib):
            """One-shot attV for y-block ib of head h: accumulate over all
            key strips jb<=4*ib+3 in a single [65,512] PSUM tile, then
            normalize into yT."""
            dt_ = h // 2
            ro = D * (h % 2)
            py = pq.tile([D + 1, 512], F32, tag="yps", bufs=2, name="yps")
            last_jb = 4 * ib + 3
            for jb in range(last_jb + 1):
                lo_g = max(512 * ib, P * jb)
                hi_g = 512 * (ib + 1)
                nc.tensor.matmul(
                    py[:, lo_g - 512 * ib:hi_g - 512 * ib],
                    lhsT=vp[:, jb, h, :],
                    rhs=att[h % 2][:, OFF_JB[jb] + lo_g - P * jb:
                                   OFF_JB[jb] + hi_g - P * jb],
                    start=(jb == 0),
                    stop=(jb == last_jb),
                    skip_group_check=True,
                )
            # denominator row -> SBUF, broadcast across 64 partitions on
            # GpSimd, fast reciprocal (~18 bits), scale y^T out of PSUM
            srow = pp.tile([1, 512], F32, tag="srow", bufs=2, name="srow")
            nc.vector.tensor_copy(srow, py[D:D + 1, :])
            sbc = pp.tile([D, 512], F32, tag="sbc", bufs=2, name="sbc")
            nc.gpsimd.partition_broadcast(sbc, srow)
            rsb = pp.tile([D, 512], F32, tag="rsb", bufs=2, name="rsb")
            nc.vector.reciprocal_approx_fast(out=rsb, in_=sbc)
            nc.vector.tensor_mul(
                out=yT[ro:ro + D, dt_, 512 * ib:512 * (ib + 1)],
                in0=py[0:D, :],
                in1=rsb,
            )

        def outproj_chunk(ib, which):
            """Project+store 256 output rows of y-block ib (2 chunks/block)."""
            tbp = 2 * ib + which
            osb = pp.tile([P, 2, C], BF16, tag="osb", bufs=3, name="osb")
            for g in range(2):
                tb = 2 * tbp + g
                for ob in range(2):
                    ps = pq.tile([P, 512], F32, tag="pp", bufs=2, name="ops")
                    for ct2 in range(DT):
                        nc.tensor.matmul(
                            ps,
                            lhsT=yT[:, ct2, tb * P:(tb + 1) * P],
                            rhs=wp_sb[:, ct2, ob * 512:(ob + 1) * 512],
                            start=(ct2 == 0),
                            stop=(ct2 == DT - 1),
                        )
                    dst = osb[:, g, ob * 512:(ob + 1) * 512]
                    if g == 1 and ob == 1:
                        nc.scalar.copy(dst, ps)  # 1/4 of casts on ACT
                    else:
                        nc.vector.tensor_copy(dst, ps)
            nc.sync.dma_start(
                out[tbp * 256:(tbp + 1) * 256, :]
                .rearrange("(g p) c -> p g c", p=P),
                osb,
            )

        # ---------------- master schedule ----------------
        # lead-in: everything attention h0 needs (Q dt0 full, K dt0 full)
        # plus V t-blocks 0-3, consumption ordered to match x-chunk DMAs
        pre = [
            ("kq", "wq", 0, 0), ("kq", "wk", 0, 0),
            ("v", 0), ("v", 1), ("v", 2), ("v", 3),
            ("kq", "wq", 0, 1), ("kq", "wk", 0, 1),
            ("kq", "wq", 0, 2), ("kq", "wk", 0, 2),
            ("kq", "wq", 0, 3), ("kq", "wk", 0, 3),
        ]
        run_unit(pre[0])
        run_unit(pre[1])
        # zero fills AFTER the first K scatter copy is queued so they
        # don't delay it on the DVE queue; kz[2]/kz[3] zeroed at h1 below
        for h in (0, 1):
            ro = D * (h % 2)
            dead = 0 if ro else D
            nc.vector.memset(kz[h][dead:dead + D, :], 0.0)
        nc.vector.memset(vp[:, :, :, D], 1.0)
        for u in pre[2:]:
            run_unit(u)

        # remaining projection units, paced into the score slots of each
        # head pass (keeps the PE fed while ACT works through the exps)
        inter = {
            0: [("v", tb) for tb in range(4, NTB)],
            1: [("kq", "wq", 1, tb) for tb in range(4)]
               + [("kq", "wk", 1, 0)],
            2: [("kq", "wk", 1, 1), ("kq", "wk", 1, 2), ("kq", "wk", 1, 3)],
            3: [],
        }
        for h in range(HPC):
            if h == 1:
                for hz in (2, 3):
                    ro = D * (hz % 2)
                    dead = 0 if ro else D
                    nc.vector.memset(kz[hz][dead:dead + D, :], 0.0)
            units = inter[h]
            stride = (NTB // len(units)) if units else 0
            ui = 0
            for jb in range(NTB):
                if units and ui < len(units) and stride and jb % stride == 0:
                    run_unit(units[ui])
                    ui += 1
                scores(h, jb)
                # bursts lag one strip so the exp of their last strip has
                # PE work (next scores) to hide behind
                if jb in (4, 8, 12):
                    ib = jb // 4 - 1
                    burst(h, ib)
                    if h == HPC - 1:
                        outproj_chunk(ib, 0)
                elif jb in (5, 9, 13) and h == HPC - 1:
                    outproj_chunk(jb // 4 - 1, 1)
            burst(h, 3)
            if h == HPC - 1:
                outproj_chunk(3, 0)
                outproj_chunk(3, 1)


def build_program(num_devices=NCORES):
    nc = bacc.Bacc(
        "TRN2",
        target_bir_lowering=False,
        debug=False,
        num_devices=num_devices,
    )
    with tile.TileContext(nc) as tc:
        _emit(nc, tc)
    nc.compile()
    return nc


_PROGRAM = None


def _get_program():
    global _PROGRAM
    if _PROGRAM is None:
        _PROGRAM = build_program()
    return _PROGRAM


def make_in_maps(x, Wk, Wq, Wv, Wp):
    bf = ml_dtypes.bfloat16
    mask = np.triu(np.ones((P, P), np.float32)).astype(bf)
    in_maps = []
    for core in range(NCORES):
        b, g = divmod(core, HPC)
        rows = slice(CS * g, CS * (g + 1))
        in_maps.append({
            "xT": np.ascontiguousarray(x[b].T).astype(bf),
            "wqT": (np.ascontiguousarray(Wq[rows].T)
                    * np.float32(0.125)).astype(bf),
            "wkT": np.ascontiguousarray(Wk[rows].T).astype(bf),
            "wvT": np.ascontiguousarray(Wv[rows].T).astype(bf),
            "wpT": np.ascontiguousarray(Wp[:, rows].T).astype(bf),
            "mask": mask,
        })
    return in_maps


def kernel(x, Wk, Wq, Wv, Wp, bp):
    global LAST_RESULTS
    x = np.asarray(x, dtype=np.float32)
    Wk = np.asarray(Wk, dtype=np.float32)
    Wq = np.asarray(Wq, dtype=np.float32)
    Wv = np.asarray(Wv, dtype=np.float32)
    Wp = np.asarray(Wp, dtype=np.float32)
    bp = np.asarray(bp, dtype=np.float32)

    nc = _get_program()
    res = run_bass_kernel_spmd(
        nc, make_in_maps(x, Wk, Wq, Wv, Wp), core_ids=list(range(NCORES))
    )
    LAST_RESULTS = res

    out = np.zeros((B, T, C), np.float64)
    for core in range(NCORES):
        out[core // HPC] += res.results[core]["out"].astype(np.float64)
    out += bp.astype(np.float64)[None, None, :]
    return out.astype(np.float32)
